# revision 1
# baseline (speedup 1.0000x reference)
"""Trainium2 Bass kernel for nn_HGNNEncoder (gnn_message_passing).

8-core SPMD: bonds and atoms sharded contiguously across cores; the f16
message / atom-message tables are AllGather-replicated each hop so the
random-index gathers stay core-local (HBM gathers via indirect DMA).

Self-contained: hardcodes the problem shapes from spec.json.
"""
import numpy as np

import concourse.bass as bass
import concourse.mybir as mybir
import concourse.tile as tile
from concourse import bacc
from concourse.bass import IndirectOffsetOnAxis
from concourse.bass_utils import run_bass_kernel_spmd
from concourse.masks import make_identity

P = 128
H = 128
NB = 6
DEPTH = 4
NCORES = 8

F32 = mybir.dt.float32
F16 = mybir.dt.float16
I32 = mybir.dt.int32


def build_nc(A, B, AF, S):
    """Build the SPMD Bass program (identical on all cores)."""
    As = A // NCORES            # atoms per core
    Bs = B // NCORES            # bonds per core
    nblkA = As // P             # atom blocks
    nblkB = Bs // P             # bond blocks
    Ms = As // S                # molecules per core
    MPB = P // S                # molecules per 128-atom block

    nc = bacc.Bacc("TRN2", target_bir_lowering=False, num_devices=NCORES)

    # ---------------- I/O ----------------
    fb = nc.dram_tensor("fb", [Bs, 147], F32, kind="ExternalInput")
    fa = nc.dram_tensor("fa", [As, 134], F32, kind="ExternalInput")  # f_atoms + ones col
    idxA = nc.dram_tensor("idxA", [P, nblkA * NB], I32, kind="ExternalInput")
    idxR = nc.dram_tensor("idxR", [P, nblkB], I32, kind="ExternalInput")
    idxB = nc.dram_tensor("idxB", [P, nblkB], I32, kind="ExternalInput")
    w_i = nc.dram_tensor("w_i", [147, H], F32, kind="ExternalInput")
    w_h = nc.dram_tensor("w_h", [H, H], F16, kind="ExternalInput")
    w_o = nc.dram_tensor("w_o", [262, H], F32, kind="ExternalInput")  # b_o folded at row 133
    w_a = nc.dram_tensor("w_a", [H, H], F32, kind="ExternalInput")
    w_b = nc.dram_tensor("w_b", [H, H], F32, kind="ExternalInput")
    amask = nc.dram_tensor("amask", [P, P], F32, kind="ExternalInput")  # additive softmax mask
    gsel = nc.dram_tensor("gsel", [P, MPB], F32, kind="ExternalInput")  # mol selector / S

    mv = nc.dram_tensor("mv", [Ms, H], F32, kind="ExternalOutput")

    # ---------------- internals ----------------
    inputs_d = nc.dram_tensor("inputs_d", [Bs, H], F32, kind="Internal")
    m_sh = [nc.dram_tensor(f"m_sh{i}", [Bs, H], F16, kind="Internal") for i in range(2)]
    am_sh = nc.dram_tensor("am_sh", [As, H], F16, kind="Internal")
    m_full = [nc.dram_tensor(f"m_full{i}", [B, H], F16, kind="Internal",
                             addr_space="Shared") for i in range(2)]
    am_full = nc.dram_tensor("am_full", [A, H], F16, kind="Internal",
                             addr_space="Shared")

    RG = [list(range(NCORES))]

    with tile.TileContext(nc) as tc:
        with tc.tile_pool(name="const", bufs=1) as cp, \
             tc.tile_pool(name="gath", bufs=16) as gp, \
             tc.tile_pool(name="work", bufs=6) as wp, \
             tc.tile_pool(name="stage", bufs=3) as sp, \
             tc.tile_pool(name="psum", bufs=2, space="PSUM") as pp, \
             tc.tile_pool(name="psum2", bufs=2, space="PSUM") as pp2:

            # constants
            id32 = cp.tile([P, P], F32)
            make_identity(nc, id32[:])
            id16 = cp.tile([P, P], F16)
            nc.vector.tensor_copy(id16[:], id32[:])
            wi_t = cp.tile([P, H], F32, tag="wi1")
            nc.sync.dma_start(out=wi_t[:], in_=w_i[0:128, :])
            wi2_t = cp.tile([P, H], F32, tag="wi2")
            nc.sync.dma_start(out=wi2_t[:19, :], in_=w_i[128:147, :])
            wh_t = cp.tile([P, H], F16, tag="wh")
            nc.sync.dma_start(out=wh_t[:], in_=w_h[:])
            wo1_t = cp.tile([P, H], F32, tag="wo1")
            nc.sync.dma_start(out=wo1_t[:], in_=w_o[0:128, :])
            wo2_t = cp.tile([P, H], F32, tag="wo2")
            nc.sync.dma_start(out=wo2_t[:6, :], in_=w_o[128:134, :])
            wo3_t = cp.tile([P, H], F32, tag="wo3")
            nc.sync.dma_start(out=wo3_t[:], in_=w_o[134:262, :])
            wa_t = cp.tile([P, H], F32, tag="wa")
            nc.sync.dma_start(out=wa_t[:], in_=w_a[:])
            wb_t = cp.tile([P, H], F32, tag="wb")
            nc.sync.dma_start(out=wb_t[:], in_=w_b[:])
            mask_t = cp.tile([P, P], F32, tag="mask")
            nc.sync.dma_start(out=mask_t[:], in_=amask[:])
            g_t = cp.tile([P, MPB], F32, tag="gsel")
            nc.sync.dma_start(out=g_t[:], in_=gsel[:])
            ixA_t = cp.tile([P, nblkA * NB], I32, tag="ixA")
            nc.sync.dma_start(out=ixA_t[:], in_=idxA[:])
            ixR_t = cp.tile([P, nblkB], I32, tag="ixR")
            nc.sync.dma_start(out=ixR_t[:], in_=idxR[:])
            ixB_t = cp.tile([P, nblkB], I32, tag="ixB")
            nc.sync.dma_start(out=ixB_t[:], in_=idxB[:])

            # ---------------- phase 0: inputs = fb @ W_i; m0 = relu ----------------
            for blk in range(nblkB):
                r0, r1 = blk * P, (blk + 1) * P
                fb_t = wp.tile([P, 147], F32, tag="fb")
                nc.sync.dma_start(out=fb_t[:], in_=fb[r0:r1, :])
                pt1 = pp.tile([P, P], F32, tag="tp")
                nc.tensor.transpose(pt1[:], fb_t[:, 0:128], id32[:])
                t1 = wp.tile([P, P], F32, tag="t1")
                nc.vector.tensor_copy(t1[:], pt1[:])
                pt2 = pp.tile([P, P], F32, tag="tp")
                nc.tensor.transpose(pt2[:19, :], fb_t[:, 128:147], id32[:])
                t2 = wp.tile([P, P], F32, tag="t2")
                nc.vector.tensor_copy(t2[:19, :], pt2[:19, :])
                pm = pp2.tile([P, P], F32, tag="mm")
                nc.tensor.matmul(pm[:], lhsT=t1[:], rhs=wi_t[:], start=True, stop=False)
                nc.tensor.matmul(pm[:], lhsT=t2[:19, :128], rhs=wi2_t[:19, :],
                                 start=False, stop=True)
                inp_t = wp.tile([P, H], F32, tag="inp")
                nc.vector.tensor_copy(inp_t[:], pm[:])
                nc.sync.dma_start(out=inputs_d[r0:r1, :], in_=inp_t[:])
                m0_t = wp.tile([P, H], F16, tag="m0")
                nc.scalar.activation(m0_t[:], inp_t[:], mybir.ActivationFunctionType.Relu)
                nc.sync.dma_start(out=m_sh[0][r0:r1, :], in_=m0_t[:])
            nc.gpsimd.collective_compute(
                "AllGather", mybir.AluOpType.bypass, replica_groups=RG,
                ins=[m_sh[0][:]], outs=[m_full[0][:]])

            # ---------------- message-passing iterations ----------------
            for t in range(1, DEPTH):
                mf = m_full[(t + 1) % 2]
                mt = m_full[t % 2]
                msh = m_sh[t % 2]
                # atom phase: am = sum_j mf[a2b[a, j]]
                for blk in range(nblkA):
                    gs = []
                    for j in range(NB):
                        g = gp.tile([P, H], F16, tag=f"g{j}")
                        nc.gpsimd.indirect_dma_start(
                            out=g[:], out_offset=None, in_=mf[:],
                            in_offset=IndirectOffsetOnAxis(
                                ap=ixA_t[:, blk * NB + j:blk * NB + j + 1], axis=0))
                        gs.append(g)
                    a01 = wp.tile([P, H], F32, tag="a01")
                    nc.vector.tensor_add(a01[:], gs[0][:], gs[1][:])
                    a23 = wp.tile([P, H], F32, tag="a23")
                    nc.vector.tensor_add(a23[:], gs[2][:], gs[3][:])
                    a45 = wp.tile([P, H], F32, tag="a45")
                    nc.vector.tensor_add(a45[:], gs[4][:], gs[5][:])
                    s1 = wp.tile([P, H], F32, tag="s1")
                    nc.vector.tensor_add(s1[:], a01[:], a23[:])
                    am16 = wp.tile([P, H], F16, tag="am16")
                    nc.vector.tensor_add(am16[:], s1[:], a45[:])
                    nc.sync.dma_start(out=am_sh[blk * P:(blk + 1) * P, :], in_=am16[:])
                nc.gpsimd.collective_compute(
                    "AllGather", mybir.AluOpType.bypass, replica_groups=RG,
                    ins=[am_sh[:]], outs=[am_full[:]])
                # bond phase: m_t = relu(inputs + (am[b2a] - mf[rev]) @ W_h)
                for blk in range(nblkB):
                    r0, r1 = blk * P, (blk + 1) * P
                    gb = gp.tile([P, H], F16, tag="gb")
                    nc.gpsimd.indirect_dma_start(
                        out=gb[:], out_offset=None, in_=am_full[:],
                        in_offset=IndirectOffsetOnAxis(
                            ap=ixB_t[:, blk:blk + 1], axis=0))
                    gr = gp.tile([P, H], F16, tag="gr")
                    nc.gpsimd.indirect_dma_start(
                        out=gr[:], out_offset=None, in_=mf[:],
                        in_offset=IndirectOffsetOnAxis(
                            ap=ixR_t[:, blk:blk + 1], axis=0))
                    diff = wp.tile([P, H], F16, tag="diff")
                    nc.vector.tensor_sub(diff[:], gb[:], gr[:])
                    pdt = pp.tile([P, H], F16, tag="tp16")
                    nc.tensor.transpose(pdt[:], diff[:], id16[:])
                    dT = wp.tile([P, H], F16, tag="dT")
                    nc.vector.tensor_copy(dT[:], pdt[:])
                    pmm = pp2.tile([P, P], F32, tag="mm")
                    nc.tensor.matmul(pmm[:], lhsT=dT[:], rhs=wh_t[:], start=True, stop=True)
                    inp_t = wp.tile([P, H], F32, tag="inp")
                    nc.sync.dma_start(out=inp_t[:], in_=inputs_d[r0:r1, :])
                    pre = wp.tile([P, H], F32, tag="pre")
                    nc.vector.tensor_add(pre[:], pmm[:], inp_t[:])
                    mt_t = wp.tile([P, H], F16, tag="mt")
                    nc.scalar.activation(mt_t[:], pre[:], mybir.ActivationFunctionType.Relu)
                    nc.sync.dma_start(out=msh[r0:r1, :], in_=mt_t[:])
                nc.gpsimd.collective_compute(
                    "AllGather", mybir.AluOpType.bypass, replica_groups=RG,
                    ins=[msh[:]], outs=[mt[:]])

            # ---------------- final: atom_hiddens + per-molecule attention ----------------
            mf = m_full[(DEPTH - 1) % 2]
            for blk in range(nblkA):
                gs = []
                for j in range(NB):
                    g = gp.tile([P, H], F16, tag=f"g{j}")
                    nc.gpsimd.indirect_dma_start(
                        out=g[:], out_offset=None, in_=mf[:],
                        in_offset=IndirectOffsetOnAxis(
                            ap=ixA_t[:, blk * NB + j:blk * NB + j + 1], axis=0))
                    gs.append(g)
                a01 = wp.tile([P, H], F32, tag="a01")
                nc.vector.tensor_add(a01[:], gs[0][:], gs[1][:])
                a23 = wp.tile([P, H], F32, tag="a23")
                nc.vector.tensor_add(a23[:], gs[2][:], gs[3][:])
                a45 = wp.tile([P, H], F32, tag="a45")
                nc.vector.tensor_add(a45[:], gs[4][:], gs[5][:])
                s1 = wp.tile([P, H], F32, tag="s1")
                nc.vector.tensor_add(s1[:], a01[:], a23[:])
                amf = wp.tile([P, H], F32, tag="amf")
                nc.vector.tensor_add(amf[:], s1[:], a45[:])
                # a_input = [f_atoms | 1 | am] @ W_o'  (b_o folded)
                fa_t = wp.tile([P, 134], F32, tag="fa")
                nc.sync.dma_start(out=fa_t[:], in_=fa[blk * P:(blk + 1) * P, :])
                pt1 = pp.tile([P, P], F32, tag="tp")
                nc.tensor.transpose(pt1[:], fa_t[:, 0:128], id32[:])
                tf1 = wp.tile([P, P], F32, tag="t1")
                nc.vector.tensor_copy(tf1[:], pt1[:])
                pt2 = pp.tile([P, P], F32, tag="tp")
                nc.tensor.transpose(pt2[:6, :], fa_t[:, 128:134], id32[:])
                tf2 = wp.tile([P, P], F32, tag="t2")
                nc.vector.tensor_copy(tf2[:6, :], pt2[:6, :])
                pt3 = pp.tile([P, P], F32, tag="tp")
                nc.tensor.transpose(pt3[:], amf[:], id32[:])
                tf3 = wp.tile([P, P], F32, tag="t3")
                nc.vector.tensor_copy(tf3[:], pt3[:])
                ph = pp2.tile([P, P], F32, tag="mm")
                nc.tensor.matmul(ph[:], lhsT=tf1[:], rhs=wo1_t[:], start=True, stop=False)
                nc.tensor.matmul(ph[:], lhsT=tf2[:6, :128], rhs=wo2_t[:6, :],
                                 start=False, stop=False)
                nc.tensor.matmul(ph[:], lhsT=tf3[:], rhs=wo3_t[:], start=False, stop=True)
                ah = wp.tile([P, H], F32, tag="ah")
                nc.scalar.activation(ah[:], ph[:], mybir.ActivationFunctionType.Relu)

                # ---- attention readout over MPB molecules in this block ----
                phT = pp.tile([P, P], F32, tag="tp")
                nc.tensor.transpose(phT[:], ah[:], id32[:])
                hT = wp.tile([P, P], F32, tag="hT")
                nc.vector.tensor_copy(hT[:], phT[:])
                pha = pp2.tile([P, P], F32, tag="mm")
                nc.tensor.matmul(pha[:], lhsT=wa_t[:], rhs=hT[:], start=True, stop=True)
                haT = wp.tile([P, P], F32, tag="haT")
                nc.vector.tensor_copy(haT[:], pha[:])
                psc = pp2.tile([P, P], F32, tag="mm")
                nc.tensor.matmul(psc[:], lhsT=haT[:], rhs=hT[:], start=True, stop=True)
                sc = wp.tile([P, P], F32, tag="sc")
                nc.vector.tensor_add(sc[:], psc[:], mask_t[:])
                mx = wp.tile([P, 1], F32, tag="mx")
                nc.vector.reduce_max(mx[:], sc[:], axis=mybir.AxisListType.X)
                e0 = wp.tile([P, P], F32, tag="e0")
                nc.vector.tensor_scalar_sub(e0[:], sc[:], mx[:])
                e = wp.tile([P, P], F32, tag="e")
                nc.scalar.activation(e[:], e0[:], mybir.ActivationFunctionType.Exp)
                sm = wp.tile([P, 1], F32, tag="sm")
                nc.vector.reduce_sum(sm[:], e[:], axis=mybir.AxisListType.X)
                rs = wp.tile([P, 1], F32, tag="rs")
                nc.vector.reciprocal(rs[:], sm[:])
                att = wp.tile([P, P], F32, tag="att")
                nc.vector.tensor_scalar_mul(att[:], e[:], rs[:])
                paT = pp.tile([P, P], F32, tag="tp")
                nc.tensor.transpose(paT[:], att[:], id32[:])
                attT = wp.tile([P, P], F32, tag="attT")
                nc.vector.tensor_copy(attT[:], paT[:])
                pz = pp2.tile([P, P], F32, tag="mm")
                nc.tensor.matmul(pz[:], lhsT=ah[:], rhs=attT[:], start=True, stop=True)
                zT = wp.tile([P, P], F32, tag="zT")
                nc.vector.tensor_copy(zT[:], pz[:])
                pah = pp2.tile([P, P], F32, tag="mm")
                nc.tensor.matmul(pah[:], lhsT=zT[:], rhs=wb_t[:], start=True, stop=True)
                rt = wp.tile([P, H], F32, tag="rt")
                nc.scalar.activation(rt[:], pah[:], mybir.ActivationFunctionType.Relu)
                tot = wp.tile([P, H], F32, tag="tot")
                nc.vector.tensor_add(tot[:], rt[:], ah[:])
                pmv = pp2.tile([MPB, H], F32, tag="pmv")
                nc.tensor.matmul(pmv[:], lhsT=g_t[:], rhs=tot[:], start=True, stop=True)
                mvo = sp.tile([P, H], F32, tag="mvs")
                nc.vector.tensor_copy(mvo[:MPB, :], pmv[:MPB, :])
                nc.sync.dma_start(out=mv[blk * MPB:(blk + 1) * MPB, :],
                                  in_=mvo[:MPB, :])
    nc.compile()
    return nc


def host_prep(f_atoms, f_bonds, W_i, W_h, W_o, b_o, W_a, W_b, b_b,
              a2b, b2a, b2revb, mol_size, A, B, AF, S):
    """Builds per-core in_maps."""
    As, Bs = A // NCORES, B // NCORES
    nblkA, nblkB = As // P, Bs // P
    MPB = P // S

    W_op = np.concatenate([W_o[:133], b_o[None, :], W_o[133:]], axis=0).astype(np.float32)
    fa_ext = np.concatenate([f_atoms, np.ones((A, 1), np.float32)], axis=1)
    amask = np.full((P, P), -30000.0, np.float32)
    for m in range(MPB):
        amask[m * S:(m + 1) * S, m * S:(m + 1) * S] = 0.0
    gsel = np.zeros((P, MPB), np.float32)
    for m in range(MPB):
        gsel[m * S:(m + 1) * S, m] = 1.0 / S

    common = dict(
        w_i=W_i.astype(np.float32), w_h=W_h.astype(np.float16),
        w_o=W_op, w_a=W_a.astype(np.float32), w_b=W_b.astype(np.float32),
        amask=amask, gsel=gsel,
    )
    in_maps = []
    for k in range(NCORES):
        a0, b0 = k * As, k * Bs
        a2b_s = a2b[a0:a0 + As]          # [As, NB]
        idxA = np.ascontiguousarray(
            a2b_s.reshape(nblkA, P, NB).transpose(1, 0, 2).reshape(P, nblkA * NB)
        ).astype(np.int32)
        idxR = np.ascontiguousarray(
            b2revb[b0:b0 + Bs].reshape(nblkB, P).T).astype(np.int32)
        idxB = np.ascontiguousarray(
            b2a[b0:b0 + Bs].reshape(nblkB, P).T).astype(np.int32)
        in_maps.append(dict(
            fb=np.ascontiguousarray(f_bonds[b0:b0 + Bs]).astype(np.float32),
            fa=np.ascontiguousarray(fa_ext[a0:a0 + As]),
            idxA=idxA, idxR=idxR, idxB=idxB, **common))
    return in_maps


_NC_CACHE = {}


def get_nc(A, B, AF, S):
    key = (A, B, AF, S)
    if key not in _NC_CACHE:
        _NC_CACHE[key] = build_nc(A, B, AF, S)
    return _NC_CACHE[key]


def kernel(f_atoms, f_bonds, W_i, W_h, W_o, b_o, W_a, W_b, b_b,
           a2b, b2a, b2revb, mol_size):
    f_atoms = np.asarray(f_atoms, np.float32)
    f_bonds = np.asarray(f_bonds, np.float32)
    A, AF = f_atoms.shape
    B = f_bonds.shape[0]
    S = int(mol_size)
    nc = get_nc(A, B, AF, S)
    in_maps = host_prep(
        f_atoms, f_bonds, np.asarray(W_i), np.asarray(W_h), np.asarray(W_o),
        np.asarray(b_o), np.asarray(W_a), np.asarray(W_b), np.asarray(b_b),
        np.asarray(a2b), np.asarray(b2a), np.asarray(b2revb), S, A, B, AF, S)
    res = run_bass_kernel_spmd(nc, in_maps, core_ids=list(range(NCORES)))
    return np.concatenate([r["mv"] for r in res.results], axis=0)



# revision 9
# speedup vs baseline: 1.1366x; 1.1366x over previous
"""Trainium2 Bass kernel for nn_HGNNEncoder (gnn_message_passing).

8-core SPMD: bonds and atoms sharded contiguously across cores; the f16
message / atom-message tables are AllGather-replicated each hop so the
random-index gathers stay core-local.

v2: multi-column batched indirect gathers (one SWDGE instruction covers
8 atom blocks x 6 neighbors, or 16 bond blocks), f16 feature pipeline,
rev-gathers staged to DRAM so they overlap the am AllGather, group-
batched direct DMAs, engine load-spreading.

Self-contained: hardcodes the problem shapes from spec.json.
"""
import numpy as np

import concourse.bass as bass
import concourse.mybir as mybir
import concourse.tile as tile
from concourse import bacc
from concourse.bass import IndirectOffsetOnAxis
from concourse.bass_utils import run_bass_kernel_spmd
from concourse.masks import make_identity

P = 128
H = 128
NB = 6
DEPTH = 4
NCORES = 8
AG = 8    # atom blocks per gather group
BG = 16   # bond blocks per gather group

F32 = mybir.dt.float32
F16 = mybir.dt.float16
I32 = mybir.dt.int32

RELU = mybir.ActivationFunctionType.Relu
COPY = mybir.ActivationFunctionType.Copy
EXP = mybir.ActivationFunctionType.Exp


def build_nc(A, B, AF, S, no_cc=False):
    """Build the SPMD Bass program (identical on all cores)."""
    As = A // NCORES            # atoms per core
    Bs = B // NCORES            # bonds per core
    nblkA = As // P             # atom blocks
    nblkB = Bs // P             # bond blocks
    ngA = nblkA // AG           # atom gather groups
    ngB = nblkB // BG           # bond gather groups
    Ms = As // S                # molecules per core
    MPB = P // S                # molecules per 128-atom block

    nc = bacc.Bacc("TRN2", target_bir_lowering=False, num_devices=NCORES)

    # ---------------- I/O ----------------
    fb = nc.dram_tensor("fb", [Bs, 147], F16, kind="ExternalInput")
    fa = nc.dram_tensor("fa", [As, 134], F16, kind="ExternalInput")  # f_atoms + ones col
    idxA = nc.dram_tensor("idxA", [P, ngA * NB * AG], I32, kind="ExternalInput")
    idxR = nc.dram_tensor("idxR", [P, nblkB], I32, kind="ExternalInput")
    idxB = nc.dram_tensor("idxB", [P, nblkB], I32, kind="ExternalInput")
    w_i = nc.dram_tensor("w_i", [147, H], F16, kind="ExternalInput")
    w_h = nc.dram_tensor("w_h", [H, H], F16, kind="ExternalInput")
    w_o = nc.dram_tensor("w_o", [262, H], F16, kind="ExternalInput")  # b_o folded at row 133
    w_a = nc.dram_tensor("w_a", [H, H], F32, kind="ExternalInput")
    w_b = nc.dram_tensor("w_b", [H, H], F32, kind="ExternalInput")
    amask = nc.dram_tensor("amask", [P, P], F32, kind="ExternalInput")  # additive softmax mask
    gsel = nc.dram_tensor("gsel", [P, MPB], F32, kind="ExternalInput")  # mol selector / S

    mv = nc.dram_tensor("mv", [Ms, H], F32, kind="ExternalOutput")

    # ---------------- internals ----------------
    inputs_d = nc.dram_tensor("inputs_d", [Bs, H], F16, kind="Internal")
    rev_d = nc.dram_tensor("rev_d", [Bs, H], F16, kind="Internal")
    m_sh = [nc.dram_tensor(f"m_sh{i}", [Bs, H], F16, kind="Internal") for i in range(2)]
    am_sh = nc.dram_tensor("am_sh", [As, H], F16, kind="Internal")
    m_full = [nc.dram_tensor(f"m_full{i}", [B, H], F16, kind="Internal",
                             addr_space="Shared") for i in range(2)]
    am_full = nc.dram_tensor("am_full", [A, H], F16, kind="Internal",
                             addr_space="Shared")

    RG = [list(range(NCORES))]

    def allgather(src, dst):
        if no_cc:
            return
        nc.gpsimd.collective_compute(
            "AllGather", mybir.AluOpType.bypass, replica_groups=RG,
            ins=[src[:]], outs=[dst[:]])

    with tile.TileContext(nc) as tc:
        with tc.tile_pool(name="const", bufs=1) as cp, \
             tc.tile_pool(name="gathA", bufs=2) as ga, \
             tc.tile_pool(name="gathB", bufs=2) as gb_p, \
             tc.tile_pool(name="work", bufs=3) as wp, \
             tc.tile_pool(name="attn", bufs=2) as ap_, \
             tc.tile_pool(name="psum", bufs=2, space="PSUM") as pp, \
             tc.tile_pool(name="psum2", bufs=2, space="PSUM") as pp2:

            # constants
            id32 = cp.tile([P, P], F32)
            make_identity(nc, id32[:])
            id16 = cp.tile([P, P], F16)
            nc.vector.tensor_copy(id16[:], id32[:])
            wi_t = cp.tile([P, H], F16, tag="wi1")
            nc.sync.dma_start(out=wi_t[:], in_=w_i[0:128, :])
            wi2_t = cp.tile([P, H], F16, tag="wi2")
            nc.sync.dma_start(out=wi2_t[:19, :], in_=w_i[128:147, :])
            wh_t = cp.tile([P, H], F16, tag="wh")
            nc.sync.dma_start(out=wh_t[:], in_=w_h[:])
            wo1_t = cp.tile([P, H], F16, tag="wo1")
            nc.sync.dma_start(out=wo1_t[:], in_=w_o[0:128, :])
            wo2_t = cp.tile([P, H], F16, tag="wo2")
            nc.sync.dma_start(out=wo2_t[:6, :], in_=w_o[128:134, :])
            wo3_t = cp.tile([P, H], F16, tag="wo3")
            nc.sync.dma_start(out=wo3_t[:], in_=w_o[134:262, :])
            wa_t = cp.tile([P, H], F32, tag="wa")
            nc.sync.dma_start(out=wa_t[:], in_=w_a[:])
            wb_t = cp.tile([P, H], F32, tag="wb")
            nc.sync.dma_start(out=wb_t[:], in_=w_b[:])
            mask_t = cp.tile([P, P], F32, tag="mask")
            nc.sync.dma_start(out=mask_t[:], in_=amask[:])
            g_t = cp.tile([P, MPB], F32, tag="gsel")
            nc.sync.dma_start(out=g_t[:], in_=gsel[:])
            ixA_t = cp.tile([P, ngA * NB * AG], I32, tag="ixA")
            nc.sync.dma_start(out=ixA_t[:], in_=idxA[:])
            ixR_t = cp.tile([P, nblkB], I32, tag="ixR")
            nc.sync.dma_start(out=ixR_t[:], in_=idxR[:])
            ixB_t = cp.tile([P, nblkB], I32, tag="ixB")
            nc.sync.dma_start(out=ixB_t[:], in_=idxB[:])

            # ---------------- phase 0: inputs = fb @ W_i; m0 = relu ----------------
            # process 4 bond blocks per group DMA
            PG = 4
            for g in range(nblkB // PG):
                r0 = g * PG * P
                fb_t = wp.tile([P, PG * 147], F16, tag="fb")
                nc.sync.dma_start(
                    out=fb_t[:].rearrange("p (bb f) -> p bb f", bb=PG),
                    in_=fb[r0:r0 + PG * P, :].rearrange(
                        "(bb p) f -> p bb f", bb=PG, p=P))
                inp_g = wp.tile([P, PG * H], F16, tag="inpg")
                m0_g = wp.tile([P, PG * H], F16, tag="m0g")
                for i in range(PG):
                    fcol = i * 147
                    pt1 = pp.tile([P, P], F16, tag="tp")
                    nc.tensor.transpose(pt1[:], fb_t[:, fcol:fcol + 128], id16[:])
                    t1 = wp.tile([P, P], F16, tag="t1")
                    nc.scalar.activation(t1[:], pt1[:], COPY)
                    pt2 = pp.tile([P, P], F16, tag="tp")
                    nc.tensor.transpose(pt2[:19, :], fb_t[:, fcol + 128:fcol + 147],
                                        id16[:])
                    t2 = wp.tile([P, P], F16, tag="t2")
                    nc.scalar.activation(t2[:19, :], pt2[:19, :], COPY)
                    pm = pp2.tile([P, P], F32, tag="mm")
                    nc.tensor.matmul(pm[:], lhsT=t1[:], rhs=wi_t[:],
                                     start=True, stop=False)
                    nc.tensor.matmul(pm[:], lhsT=t2[:19, :128], rhs=wi2_t[:19, :],
                                     start=False, stop=True)
                    nc.vector.tensor_copy(inp_g[:, i * H:(i + 1) * H], pm[:])
                    nc.scalar.activation(m0_g[:, i * H:(i + 1) * H], pm[:], RELU)
                nc.sync.dma_start(
                    out=inputs_d[r0:r0 + PG * P, :].rearrange(
                        "(bb p) h -> p bb h", bb=PG, p=P),
                    in_=inp_g[:].rearrange("p (bb h) -> p bb h", bb=PG))
                nc.scalar.dma_start(
                    out=m_sh[0][r0:r0 + PG * P, :].rearrange(
                        "(bb p) h -> p bb h", bb=PG, p=P),
                    in_=m0_g[:].rearrange("p (bb h) -> p bb h", bb=PG))
            allgather(m_sh[0], m_full[0])

            def atom_phase(mf, out_groups):
                """out_groups: callback(g, am8_tile) for each atom group."""
                for g in range(ngA):
                    c0 = g * NB * AG
                    g48 = ga.tile([P, NB * AG * H], F16, tag="g48")
                    for c in range(NB * AG):
                        nc.gpsimd.indirect_dma_start(
                            out=g48[:, c * H:(c + 1) * H], out_offset=None,
                            in_=mf[:],
                            in_offset=IndirectOffsetOnAxis(
                                ap=ixA_t[:, c0 + c:c0 + c + 1], axis=0))
                    W = AG * H
                    s1 = wp.tile([P, W], F16, tag="s1")
                    nc.vector.tensor_add(s1[:], g48[:, 0:W], g48[:, W:2 * W])
                    s2 = wp.tile([P, W], F16, tag="s2")
                    nc.vector.tensor_add(s2[:], g48[:, 2 * W:3 * W], g48[:, 3 * W:4 * W])
                    s3 = wp.tile([P, W], F16, tag="s3")
                    nc.vector.tensor_add(s3[:], g48[:, 4 * W:5 * W], g48[:, 5 * W:6 * W])
                    s12 = wp.tile([P, W], F16, tag="s12")
                    nc.vector.tensor_add(s12[:], s1[:], s2[:])
                    am8 = wp.tile([P, W], F16, tag="am8")
                    nc.vector.tensor_add(am8[:], s12[:], s3[:])
                    out_groups(g, am8)

            # ---------------- message-passing iterations ----------------
            for t in range(1, DEPTH):
                mf = m_full[(t + 1) % 2]
                mt = m_full[t % 2]
                msh = m_sh[t % 2]

                # atom phase: am = sum_j mf[a2b[a, j]]  -> am_sh
                def store_am(g, am8):
                    r0 = g * AG * P
                    nc.sync.dma_start(
                        out=am_sh[r0:r0 + AG * P, :].rearrange(
                            "(bb p) h -> p bb h", bb=AG, p=P),
                        in_=am8[:].rearrange("p (bb h) -> p bb h", bb=AG))
                atom_phase(mf, store_am)
                allgather(am_sh, am_full)

                # rev staging: rev_d[b] = mf[b2revb[b]]; overlaps am AllGather
                for g in range(ngB):
                    gr16 = gb_p.tile([P, BG * H], F16, tag="gr16")
                    for c in range(BG):
                        nc.gpsimd.indirect_dma_start(
                            out=gr16[:, c * H:(c + 1) * H], out_offset=None,
                            in_=mf[:],
                            in_offset=IndirectOffsetOnAxis(
                                ap=ixR_t[:, g * BG + c:g * BG + c + 1], axis=0))
                    r0 = g * BG * P
                    nc.scalar.dma_start(
                        out=rev_d[r0:r0 + BG * P, :].rearrange(
                            "(bb p) h -> p bb h", bb=BG, p=P),
                        in_=gr16[:].rearrange("p (bb h) -> p bb h", bb=BG))

                # bond phase: m_t = relu(inputs + (am[b2a] - mf[rev]) @ W_h)
                for g in range(ngB):
                    r0 = g * BG * P
                    gb16 = gb_p.tile([P, BG * H], F16, tag="gb16")
                    for c in range(BG):
                        nc.gpsimd.indirect_dma_start(
                            out=gb16[:, c * H:(c + 1) * H], out_offset=None,
                            in_=am_full[:],
                            in_offset=IndirectOffsetOnAxis(
                                ap=ixB_t[:, g * BG + c:g * BG + c + 1], axis=0))
                    grl = gb_p.tile([P, BG * H], F16, tag="grl")
                    nc.sync.dma_start(
                        out=grl[:].rearrange("p (bb h) -> p bb h", bb=BG),
                        in_=rev_d[r0:r0 + BG * P, :].rearrange(
                            "(bb p) h -> p bb h", bb=BG, p=P))
                    diff16 = gb_p.tile([P, BG * H], F16, tag="diff16")
                    nc.vector.tensor_sub(diff16[:], gb16[:], grl[:])
                    inp_g = wp.tile([P, BG * H], F16, tag="binp")
                    nc.sync.dma_start(
                        out=inp_g[:].rearrange("p (bb h) -> p bb h", bb=BG),
                        in_=inputs_d[r0:r0 + BG * P, :].rearrange(
                            "(bb p) h -> p bb h", bb=BG, p=P))
                    pre_g = wp.tile([P, BG * H], F16, tag="bpre")
                    for i in range(BG):
                        cl = slice(i * H, (i + 1) * H)
                        pdt = pp.tile([P, H], F16, tag="tp")
                        nc.tensor.transpose(pdt[:], diff16[:, cl], id16[:])
                        dT = wp.tile([P, H], F16, tag="dT")
                        nc.scalar.activation(dT[:], pdt[:], COPY)
                        pmm = pp2.tile([P, P], F32, tag="mm")
                        nc.tensor.matmul(pmm[:], lhsT=dT[:], rhs=wh_t[:],
                                         start=True, stop=True)
                        nc.vector.tensor_add(pre_g[:, cl], pmm[:], inp_g[:, cl])
                    mt_g = wp.tile([P, BG * H], F16, tag="bmt")
                    nc.scalar.activation(mt_g[:], pre_g[:], RELU)
                    nc.scalar.dma_start(
                        out=msh[r0:r0 + BG * P, :].rearrange(
                            "(bb p) h -> p bb h", bb=BG, p=P),
                        in_=mt_g[:].rearrange("p (bb h) -> p bb h", bb=BG))
                allgather(msh, mt)

            # ---------------- final: atom_hiddens + per-molecule attention ----------------
            mf = m_full[(DEPTH - 1) % 2]

            def final_group(g, am8):
                r0 = g * AG * P
                fa_g = wp.tile([P, AG * 134], F16, tag="fag")
                nc.sync.dma_start(
                    out=fa_g[:].rearrange("p (bb f) -> p bb f", bb=AG),
                    in_=fa[r0:r0 + AG * P, :].rearrange(
                        "(bb p) f -> p bb f", bb=AG, p=P))
                mv_g = ap_.tile([P, AG * H], F32, tag="mvg")
                for i in range(AG):
                    fcol = i * 134
                    pt1 = pp.tile([P, P], F16, tag="tp")
                    nc.tensor.transpose(pt1[:], fa_g[:, fcol:fcol + 128], id16[:])
                    tf1 = wp.tile([P, P], F16, tag="t1")
                    nc.scalar.activation(tf1[:], pt1[:], COPY)
                    pt2 = pp.tile([P, P], F16, tag="tp")
                    nc.tensor.transpose(pt2[:6, :], fa_g[:, fcol + 128:fcol + 134],
                                        id16[:])
                    tf2 = wp.tile([P, P], F16, tag="t2")
                    nc.scalar.activation(tf2[:6, :], pt2[:6, :], COPY)
                    pt3 = pp.tile([P, P], F16, tag="tp")
                    nc.tensor.transpose(pt3[:], am8[:, i * H:(i + 1) * H], id16[:])
                    tf3 = wp.tile([P, P], F16, tag="t3")
                    nc.scalar.activation(tf3[:], pt3[:], COPY)
                    ph = pp2.tile([P, P], F32, tag="mm")
                    nc.tensor.matmul(ph[:], lhsT=tf1[:], rhs=wo1_t[:],
                                     start=True, stop=False)
                    nc.tensor.matmul(ph[:], lhsT=tf2[:6, :128], rhs=wo2_t[:6, :],
                                     start=False, stop=False)
                    nc.tensor.matmul(ph[:], lhsT=tf3[:], rhs=wo3_t[:],
                                     start=False, stop=True)
                    ah = ap_.tile([P, H], F32, tag="ah")
                    nc.scalar.activation(ah[:], ph[:], RELU)

                    # ---- attention readout over MPB molecules in this block ----
                    phT = pp.tile([P, P], F32, tag="tpf")
                    nc.tensor.transpose(phT[:], ah[:], id32[:])
                    hT = ap_.tile([P, P], F32, tag="hT")
                    nc.scalar.activation(hT[:], phT[:], COPY)
                    pha = pp2.tile([P, P], F32, tag="mm")
                    nc.tensor.matmul(pha[:], lhsT=wa_t[:], rhs=hT[:],
                                     start=True, stop=True)
                    haT = ap_.tile([P, P], F32, tag="haT")
                    nc.scalar.activation(haT[:], pha[:], COPY)
                    psc = pp2.tile([P, P], F32, tag="mm")
                    nc.tensor.matmul(psc[:], lhsT=haT[:], rhs=hT[:],
                                     start=True, stop=True)
                    sc = ap_.tile([P, P], F32, tag="sc")
                    nc.vector.tensor_add(sc[:], psc[:], mask_t[:])
                    mx = ap_.tile([P, 1], F32, tag="mx")
                    nc.vector.reduce_max(mx[:], sc[:], axis=mybir.AxisListType.X)
                    e0 = ap_.tile([P, P], F32, tag="e0")
                    nc.vector.tensor_scalar_sub(e0[:], sc[:], mx[:])
                    e = ap_.tile([P, P], F32, tag="e")
                    nc.scalar.activation(e[:], e0[:], EXP)
                    sm = ap_.tile([P, 1], F32, tag="sm")
                    nc.vector.reduce_sum(sm[:], e[:], axis=mybir.AxisListType.X)
                    rs = ap_.tile([P, 1], F32, tag="rs")
                    nc.vector.reciprocal(rs[:], sm[:])
                    att = ap_.tile([P, P], F32, tag="att")
                    nc.vector.tensor_scalar_mul(att[:], e[:], rs[:])
                    paT = pp.tile([P, P], F32, tag="tpf")
                    nc.tensor.transpose(paT[:], att[:], id32[:])
                    attT = ap_.tile([P, P], F32, tag="attT")
                    nc.scalar.activation(attT[:], paT[:], COPY)
                    pz = pp2.tile([P, P], F32, tag="mm")
                    nc.tensor.matmul(pz[:], lhsT=ah[:], rhs=attT[:],
                                     start=True, stop=True)
                    zT = ap_.tile([P, P], F32, tag="zT")
                    nc.scalar.activation(zT[:], pz[:], COPY)
                    pah = pp2.tile([P, P], F32, tag="mm")
                    nc.tensor.matmul(pah[:], lhsT=zT[:], rhs=wb_t[:],
                                     start=True, stop=True)
                    rt = ap_.tile([P, H], F32, tag="rt")
                    nc.scalar.activation(rt[:], pah[:], RELU)
                    tot = ap_.tile([P, H], F32, tag="tot")
                    nc.vector.tensor_add(tot[:], rt[:], ah[:])
                    pmv = pp2.tile([MPB, H], F32, tag="pmv")
                    nc.tensor.matmul(pmv[:], lhsT=g_t[:], rhs=tot[:],
                                     start=True, stop=True)
                    nc.vector.tensor_copy(mv_g[:MPB, i * H:(i + 1) * H],
                                          pmv[:MPB, :])
                # mv rows for this group: g*AG*MPB .. +AG*MPB, MPB rows per block
                nc.sync.dma_start(
                    out=mv[g * AG * MPB:(g + 1) * AG * MPB, :].rearrange(
                        "(bb m) h -> m bb h", bb=AG, m=MPB),
                    in_=mv_g[:MPB, :AG * H].rearrange(
                        "m (bb h) -> m bb h", bb=AG))
            atom_phase(mf, final_group)
    nc.compile()
    return nc


def host_prep(f_atoms, f_bonds, W_i, W_h, W_o, b_o, W_a, W_b, b_b,
              a2b, b2a, b2revb, mol_size, A, B, AF, S):
    """Builds per-core in_maps."""
    As, Bs = A // NCORES, B // NCORES
    nblkA, nblkB = As // P, Bs // P
    ngA = nblkA // AG
    MPB = P // S

    W_op = np.concatenate([W_o[:133], b_o[None, :], W_o[133:]],
                          axis=0).astype(np.float16)
    fa_ext = np.concatenate([np.asarray(f_atoms, np.float32),
                             np.ones((A, 1), np.float32)], axis=1).astype(np.float16)
    fb16 = np.asarray(f_bonds, np.float16)
    amask = np.full((P, P), -30000.0, np.float32)
    for m in range(MPB):
        amask[m * S:(m + 1) * S, m * S:(m + 1) * S] = 0.0
    gsel = np.zeros((P, MPB), np.float32)
    for m in range(MPB):
        gsel[m * S:(m + 1) * S, m] = 1.0 / S

    common = dict(
        w_i=np.asarray(W_i, np.float16), w_h=np.asarray(W_h, np.float16),
        w_o=W_op, w_a=np.asarray(W_a, np.float32), w_b=np.asarray(W_b, np.float32),
        amask=amask, gsel=gsel,
    )
    in_maps = []
    for k in range(NCORES):
        a0, b0 = k * As, k * Bs
        a2b_s = a2b[a0:a0 + As]          # [As, NB]
        # column layout per atom group g: col = j*AG + bb (j-major)
        idxA = np.ascontiguousarray(
            a2b_s.reshape(ngA, AG, P, NB).transpose(2, 0, 3, 1).reshape(
                P, ngA * NB * AG)).astype(np.int32)
        idxR = np.ascontiguousarray(
            b2revb[b0:b0 + Bs].reshape(nblkB, P).T).astype(np.int32)
        idxB = np.ascontiguousarray(
            b2a[b0:b0 + Bs].reshape(nblkB, P).T).astype(np.int32)
        in_maps.append(dict(
            fb=np.ascontiguousarray(fb16[b0:b0 + Bs]),
            fa=np.ascontiguousarray(fa_ext[a0:a0 + As]),
            idxA=idxA, idxR=idxR, idxB=idxB, **common))
    return in_maps


_NC_CACHE = {}


def get_nc(A, B, AF, S, no_cc=False):
    key = (A, B, AF, S, no_cc)
    if key not in _NC_CACHE:
        _NC_CACHE[key] = build_nc(A, B, AF, S, no_cc=no_cc)
    return _NC_CACHE[key]


def kernel(f_atoms, f_bonds, W_i, W_h, W_o, b_o, W_a, W_b, b_b,
           a2b, b2a, b2revb, mol_size):
    f_atoms = np.asarray(f_atoms, np.float32)
    f_bonds = np.asarray(f_bonds, np.float32)
    A, AF = f_atoms.shape
    B = f_bonds.shape[0]
    S = int(mol_size)
    nc = get_nc(A, B, AF, S)
    in_maps = host_prep(
        f_atoms, f_bonds, np.asarray(W_i), np.asarray(W_h), np.asarray(W_o),
        np.asarray(b_o), np.asarray(W_a), np.asarray(W_b), np.asarray(b_b),
        np.asarray(a2b), np.asarray(b2a), np.asarray(b2revb), S, A, B, AF, S)
    res = run_bass_kernel_spmd(nc, in_maps, core_ids=list(range(NCORES)))
    return np.concatenate([r["mv"] for r in res.results], axis=0)


# revision 13
# speedup vs baseline: 1.1752x; 1.0340x over previous
"""Trainium2 Bass kernel for nn_HGNNEncoder (gnn_message_passing).

8-core SPMD: bonds and atoms sharded contiguously across cores; the f16
message / atom-message tables are AllGather-replicated each hop so the
random-index gathers stay core-local.

v2: multi-column batched indirect gathers (one SWDGE instruction covers
8 atom blocks x 6 neighbors, or 16 bond blocks), f16 feature pipeline,
rev-gathers staged to DRAM so they overlap the am AllGather, group-
batched direct DMAs, engine load-spreading.

Self-contained: hardcodes the problem shapes from spec.json.
"""
import numpy as np

import concourse.bass as bass
import concourse.mybir as mybir
import concourse.tile as tile
from concourse import bacc
from concourse.bass import IndirectOffsetOnAxis
from concourse.bass_utils import run_bass_kernel_spmd
from concourse.masks import make_identity

P = 128
H = 128
NB = 6
DEPTH = 4
NCORES = 8
AG = 8    # atom blocks per gather group
BG = 16   # bond blocks per gather group

F32 = mybir.dt.float32
F16 = mybir.dt.float16
I32 = mybir.dt.int32

RELU = mybir.ActivationFunctionType.Relu
COPY = mybir.ActivationFunctionType.Copy
EXP = mybir.ActivationFunctionType.Exp


def build_nc(A, B, AF, S, no_cc=False, no_gather=False):
    """Build the SPMD Bass program (identical on all cores)."""
    As = A // NCORES            # atoms per core
    Bs = B // NCORES            # bonds per core
    nblkA = As // P             # atom blocks
    nblkB = Bs // P             # bond blocks
    ngA = nblkA // AG           # atom gather groups
    ngB = nblkB // BG           # bond gather groups
    Ms = As // S                # molecules per core
    MPB = P // S                # molecules per 128-atom block

    nc = bacc.Bacc("TRN2", target_bir_lowering=False, num_devices=NCORES)

    # ---------------- I/O ----------------
    fb = nc.dram_tensor("fb", [Bs, 147], F16, kind="ExternalInput")
    fa = nc.dram_tensor("fa", [As, 134], F16, kind="ExternalInput")  # f_atoms + ones col
    idxA = nc.dram_tensor("idxA", [P, ngA * NB * AG], I32, kind="ExternalInput")
    idxR = nc.dram_tensor("idxR", [P, nblkB], I32, kind="ExternalInput")
    idxB = nc.dram_tensor("idxB", [P, nblkB], I32, kind="ExternalInput")
    w_i = nc.dram_tensor("w_i", [147, H], F16, kind="ExternalInput")
    w_h = nc.dram_tensor("w_h", [H, H], F16, kind="ExternalInput")
    w_o = nc.dram_tensor("w_o", [262, H], F16, kind="ExternalInput")  # b_o folded at row 133
    w_a = nc.dram_tensor("w_a", [H, H], F32, kind="ExternalInput")
    w_b = nc.dram_tensor("w_b", [H, H], F32, kind="ExternalInput")
    amask = nc.dram_tensor("amask", [P, P], F32, kind="ExternalInput")  # additive softmax mask
    gsel = nc.dram_tensor("gsel", [P, MPB], F32, kind="ExternalInput")  # mol selector / S

    mv = nc.dram_tensor("mv", [Ms, H], F32, kind="ExternalOutput")

    # ---------------- internals ----------------
    inputs_d = nc.dram_tensor("inputs_d", [Bs, H], F16, kind="Internal")
    rev_d = nc.dram_tensor("rev_d", [Bs, H], F16, kind="Internal")
    m_sh = [nc.dram_tensor(f"m_sh{i}", [Bs, H], F16, kind="Internal") for i in range(2)]
    am_sh = nc.dram_tensor("am_sh", [As, H], F16, kind="Internal")
    m_full = [nc.dram_tensor(f"m_full{i}", [B, H], F16, kind="Internal",
                             addr_space="Shared") for i in range(2)]
    am_full = nc.dram_tensor("am_full", [A, H], F16, kind="Internal",
                             addr_space="Shared")

    RG = [list(range(NCORES))]

    def igather(**kw):
        if no_gather:
            return
        nc.gpsimd.indirect_dma_start(**kw)

    def fill_if_nogather(t):
        if no_gather:
            nc.vector.memset(t[:], 0)

    NCH = 4                      # m-allgather chunks

    def allgather(src, dst):
        if no_cc:
            return
        nc.gpsimd.collective_compute(
            "AllGather", mybir.AluOpType.bypass, replica_groups=RG,
            ins=[src[:]], outs=[dst[:]])

    def allgather_m(src, dst):
        # chunk-major dst layout: [chunk][core][Bs/NCH rows]
        if no_cc:
            return
        csz = Bs // NCH
        for c in range(NCH):
            nc.gpsimd.collective_compute(
                "AllGather", mybir.AluOpType.bypass, replica_groups=RG,
                ins=[src[c * csz:(c + 1) * csz, :]],
                outs=[dst[c * csz * NCORES:(c + 1) * csz * NCORES, :]])

    with tile.TileContext(nc) as tc:
        with tc.tile_pool(name="const", bufs=1) as cp, \
             tc.tile_pool(name="gathA", bufs=2) as ga, \
             tc.tile_pool(name="gathB", bufs=2) as gb_p, \
             tc.tile_pool(name="work", bufs=3) as wp, \
             tc.tile_pool(name="attn", bufs=2) as ap_, \
             tc.tile_pool(name="psum", bufs=2, space="PSUM") as pp, \
             tc.tile_pool(name="psum2", bufs=2, space="PSUM") as pp2:

            # constants
            id32 = cp.tile([P, P], F32)
            make_identity(nc, id32[:])
            id16 = cp.tile([P, P], F16)
            nc.vector.tensor_copy(id16[:], id32[:])
            wi_t = cp.tile([P, H], F16, tag="wi1")
            nc.sync.dma_start(out=wi_t[:], in_=w_i[0:128, :])
            wi2_t = cp.tile([P, H], F16, tag="wi2")
            nc.sync.dma_start(out=wi2_t[:19, :], in_=w_i[128:147, :])
            wh_t = cp.tile([P, H], F16, tag="wh")
            nc.sync.dma_start(out=wh_t[:], in_=w_h[:])
            wo1_t = cp.tile([P, H], F16, tag="wo1")
            nc.sync.dma_start(out=wo1_t[:], in_=w_o[0:128, :])
            wo2_t = cp.tile([P, H], F16, tag="wo2")
            nc.sync.dma_start(out=wo2_t[:6, :], in_=w_o[128:134, :])
            wo3_t = cp.tile([P, H], F16, tag="wo3")
            nc.sync.dma_start(out=wo3_t[:], in_=w_o[134:262, :])
            wa_t = cp.tile([P, H], F32, tag="wa")
            nc.sync.dma_start(out=wa_t[:], in_=w_a[:])
            wb_t = cp.tile([P, H], F32, tag="wb")
            nc.sync.dma_start(out=wb_t[:], in_=w_b[:])
            mask_t = cp.tile([P, P], F32, tag="mask")
            nc.sync.dma_start(out=mask_t[:], in_=amask[:])
            g_t = cp.tile([P, MPB], F32, tag="gsel")
            nc.sync.dma_start(out=g_t[:], in_=gsel[:])
            ixA_t = cp.tile([P, ngA * NB * AG], I32, tag="ixA")
            nc.sync.dma_start(out=ixA_t[:], in_=idxA[:])
            ixR_t = cp.tile([P, nblkB], I32, tag="ixR")
            nc.sync.dma_start(out=ixR_t[:], in_=idxR[:])
            ixB_t = cp.tile([P, nblkB], I32, tag="ixB")
            nc.sync.dma_start(out=ixB_t[:], in_=idxB[:])

            # ---------------- phase 0: inputs = fb @ W_i; m0 = relu ----------------
            # process 4 bond blocks per group DMA
            PG = 4
            for g in range(nblkB // PG):
                r0 = g * PG * P
                fb_t = wp.tile([P, PG * 147], F16, tag="fb")
                nc.sync.dma_start(
                    out=fb_t[:].rearrange("p (bb f) -> p bb f", bb=PG),
                    in_=fb[r0:r0 + PG * P, :].rearrange(
                        "(bb p) f -> p bb f", bb=PG, p=P))
                inp_g = wp.tile([P, PG * H], F16, tag="inpg")
                m0_g = wp.tile([P, PG * H], F16, tag="m0g")
                for i in range(PG):
                    fcol = i * 147
                    pt1 = pp.tile([P, P], F16, tag="tp")
                    nc.tensor.transpose(pt1[:], fb_t[:, fcol:fcol + 128], id16[:])
                    t1 = wp.tile([P, P], F16, tag="t1")
                    nc.scalar.activation(t1[:], pt1[:], COPY)
                    pt2 = pp.tile([P, P], F16, tag="tp")
                    nc.tensor.transpose(pt2[:19, :], fb_t[:, fcol + 128:fcol + 147],
                                        id16[:])
                    t2 = wp.tile([P, P], F16, tag="t2")
                    nc.scalar.activation(t2[:19, :], pt2[:19, :], COPY)
                    pm = pp2.tile([P, P], F32, tag="mm")
                    nc.tensor.matmul(pm[:], lhsT=t1[:], rhs=wi_t[:],
                                     start=True, stop=False)
                    nc.tensor.matmul(pm[:], lhsT=t2[:19, :128], rhs=wi2_t[:19, :],
                                     start=False, stop=True)
                    nc.vector.tensor_copy(inp_g[:, i * H:(i + 1) * H], pm[:])
                    nc.scalar.activation(m0_g[:, i * H:(i + 1) * H], pm[:], RELU)
                nc.sync.dma_start(
                    out=inputs_d[r0:r0 + PG * P, :].rearrange(
                        "(bb p) h -> p bb h", bb=PG, p=P),
                    in_=inp_g[:].rearrange("p (bb h) -> p bb h", bb=PG))
                nc.scalar.dma_start(
                    out=m_sh[0][r0:r0 + PG * P, :].rearrange(
                        "(bb p) h -> p bb h", bb=PG, p=P),
                    in_=m0_g[:].rearrange("p (bb h) -> p bb h", bb=PG))
            allgather_m(m_sh[0], m_full[0])

            def atom_phase(mf, out_groups):
                """out_groups: callback(g, am8_tile) for each atom group."""
                for g in range(ngA):
                    c0 = g * NB * AG
                    g48 = ga.tile([P, NB * AG * H], F16, tag="g48")
                    fill_if_nogather(g48)
                    for c in range(NB * AG):
                        igather(
                            out=g48[:, c * H:(c + 1) * H], out_offset=None,
                            in_=mf[:],
                            in_offset=IndirectOffsetOnAxis(
                                ap=ixA_t[:, c0 + c:c0 + c + 1], axis=0))
                    W = AG * H
                    s1 = wp.tile([P, W], F16, tag="s1")
                    nc.vector.tensor_add(s1[:], g48[:, 0:W], g48[:, W:2 * W])
                    s2 = wp.tile([P, W], F16, tag="s2")
                    nc.vector.tensor_add(s2[:], g48[:, 2 * W:3 * W], g48[:, 3 * W:4 * W])
                    s3 = wp.tile([P, W], F16, tag="s3")
                    nc.vector.tensor_add(s3[:], g48[:, 4 * W:5 * W], g48[:, 5 * W:6 * W])
                    s12 = wp.tile([P, W], F16, tag="s12")
                    nc.vector.tensor_add(s12[:], s1[:], s2[:])
                    am8 = wp.tile([P, W], F16, tag="am8")
                    nc.vector.tensor_add(am8[:], s12[:], s3[:])
                    out_groups(g, am8)

            # ---------------- message-passing iterations ----------------
            for t in range(1, DEPTH):
                mf = m_full[(t + 1) % 2]
                mt = m_full[t % 2]
                msh = m_sh[t % 2]

                # atom phase: am = sum_j mf[a2b[a, j]]  -> am_sh
                def store_am(g, am8):
                    r0 = g * AG * P
                    nc.sync.dma_start(
                        out=am_sh[r0:r0 + AG * P, :].rearrange(
                            "(bb p) h -> p bb h", bb=AG, p=P),
                        in_=am8[:].rearrange("p (bb h) -> p bb h", bb=AG))
                atom_phase(mf, store_am)
                allgather(am_sh, am_full)

                # rev staging: rev_d[b] = mf[b2revb[b]]; overlaps am AllGather
                for g in range(ngB):
                    gr16 = gb_p.tile([P, BG * H], F16, tag="gr16")
                    fill_if_nogather(gr16)
                    for c in range(BG):
                        igather(
                            out=gr16[:, c * H:(c + 1) * H], out_offset=None,
                            in_=mf[:],
                            in_offset=IndirectOffsetOnAxis(
                                ap=ixR_t[:, g * BG + c:g * BG + c + 1], axis=0))
                    r0 = g * BG * P
                    nc.scalar.dma_start(
                        out=rev_d[r0:r0 + BG * P, :].rearrange(
                            "(bb p) h -> p bb h", bb=BG, p=P),
                        in_=gr16[:].rearrange("p (bb h) -> p bb h", bb=BG))

                # bond phase: m_t = relu(inputs + (am[b2a] - mf[rev]) @ W_h)
                for g in range(ngB):
                    r0 = g * BG * P
                    gb16 = gb_p.tile([P, BG * H], F16, tag="gb16")
                    fill_if_nogather(gb16)
                    for c in range(BG):
                        igather(
                            out=gb16[:, c * H:(c + 1) * H], out_offset=None,
                            in_=am_full[:],
                            in_offset=IndirectOffsetOnAxis(
                                ap=ixB_t[:, g * BG + c:g * BG + c + 1], axis=0))
                    grl = gb_p.tile([P, BG * H], F16, tag="grl")
                    nc.sync.dma_start(
                        out=grl[:].rearrange("p (bb h) -> p bb h", bb=BG),
                        in_=rev_d[r0:r0 + BG * P, :].rearrange(
                            "(bb p) h -> p bb h", bb=BG, p=P))
                    diff16 = gb_p.tile([P, BG * H], F16, tag="diff16")
                    nc.vector.tensor_sub(diff16[:], gb16[:], grl[:])
                    inp_g = wp.tile([P, BG * H], F16, tag="binp")
                    nc.sync.dma_start(
                        out=inp_g[:].rearrange("p (bb h) -> p bb h", bb=BG),
                        in_=inputs_d[r0:r0 + BG * P, :].rearrange(
                            "(bb p) h -> p bb h", bb=BG, p=P))
                    pre_g = wp.tile([P, BG * H], F16, tag="bpre")
                    for i in range(BG):
                        cl = slice(i * H, (i + 1) * H)
                        pdt = pp.tile([P, H], F16, tag="tp")
                        nc.tensor.transpose(pdt[:], diff16[:, cl], id16[:])
                        dT = wp.tile([P, H], F16, tag="dT")
                        nc.scalar.activation(dT[:], pdt[:], COPY)
                        pmm = pp2.tile([P, P], F32, tag="mm")
                        nc.tensor.matmul(pmm[:], lhsT=dT[:], rhs=wh_t[:],
                                         start=True, stop=True)
                        nc.vector.tensor_add(pre_g[:, cl], pmm[:], inp_g[:, cl])
                    mt_g = wp.tile([P, BG * H], F16, tag="bmt")
                    nc.scalar.activation(mt_g[:], pre_g[:], RELU)
                    nc.scalar.dma_start(
                        out=msh[r0:r0 + BG * P, :].rearrange(
                            "(bb p) h -> p bb h", bb=BG, p=P),
                        in_=mt_g[:].rearrange("p (bb h) -> p bb h", bb=BG))
                allgather_m(msh, mt)

            # ---------------- final: atom_hiddens + per-molecule attention ----------------
            mf = m_full[(DEPTH - 1) % 2]

            def final_group(g, am8):
                r0 = g * AG * P
                fa_g = wp.tile([P, AG * 134], F16, tag="fag")
                nc.sync.dma_start(
                    out=fa_g[:].rearrange("p (bb f) -> p bb f", bb=AG),
                    in_=fa[r0:r0 + AG * P, :].rearrange(
                        "(bb p) f -> p bb f", bb=AG, p=P))
                mv_g = ap_.tile([P, AG * H], F32, tag="mvg")
                for i in range(AG):
                    fcol = i * 134
                    pt1 = pp.tile([P, P], F16, tag="tp")
                    nc.tensor.transpose(pt1[:], fa_g[:, fcol:fcol + 128], id16[:])
                    tf1 = wp.tile([P, P], F16, tag="t1")
                    nc.scalar.activation(tf1[:], pt1[:], COPY)
                    pt2 = pp.tile([P, P], F16, tag="tp")
                    nc.tensor.transpose(pt2[:6, :], fa_g[:, fcol + 128:fcol + 134],
                                        id16[:])
                    tf2 = wp.tile([P, P], F16, tag="t2")
                    nc.scalar.activation(tf2[:6, :], pt2[:6, :], COPY)
                    pt3 = pp.tile([P, P], F16, tag="tp")
                    nc.tensor.transpose(pt3[:], am8[:, i * H:(i + 1) * H], id16[:])
                    tf3 = wp.tile([P, P], F16, tag="t3")
                    nc.scalar.activation(tf3[:], pt3[:], COPY)
                    ph = pp2.tile([P, P], F32, tag="mm")
                    nc.tensor.matmul(ph[:], lhsT=tf1[:], rhs=wo1_t[:],
                                     start=True, stop=False)
                    nc.tensor.matmul(ph[:], lhsT=tf2[:6, :128], rhs=wo2_t[:6, :],
                                     start=False, stop=False)
                    nc.tensor.matmul(ph[:], lhsT=tf3[:], rhs=wo3_t[:],
                                     start=False, stop=True)
                    ah = ap_.tile([P, H], F32, tag="ah")
                    nc.scalar.activation(ah[:], ph[:], RELU)

                    # ---- attention readout over MPB molecules in this block ----
                    phT = pp.tile([P, P], F32, tag="tpf")
                    nc.tensor.transpose(phT[:], ah[:], id32[:])
                    hT = ap_.tile([P, P], F32, tag="hT")
                    nc.scalar.activation(hT[:], phT[:], COPY)
                    pha = pp2.tile([P, P], F32, tag="mm")
                    nc.tensor.matmul(pha[:], lhsT=wa_t[:], rhs=hT[:],
                                     start=True, stop=True)
                    haT = ap_.tile([P, P], F32, tag="haT")
                    nc.scalar.activation(haT[:], pha[:], COPY)
                    psc = pp2.tile([P, P], F32, tag="mm")
                    nc.tensor.matmul(psc[:], lhsT=haT[:], rhs=hT[:],
                                     start=True, stop=True)
                    sc = ap_.tile([P, P], F32, tag="sc")
                    nc.vector.tensor_add(sc[:], psc[:], mask_t[:])
                    mx = ap_.tile([P, 1], F32, tag="mx")
                    nc.vector.reduce_max(mx[:], sc[:], axis=mybir.AxisListType.X)
                    e0 = ap_.tile([P, P], F32, tag="e0")
                    nc.vector.tensor_scalar_sub(e0[:], sc[:], mx[:])
                    e = ap_.tile([P, P], F32, tag="e")
                    nc.scalar.activation(e[:], e0[:], EXP)
                    sm = ap_.tile([P, 1], F32, tag="sm")
                    nc.vector.reduce_sum(sm[:], e[:], axis=mybir.AxisListType.X)
                    rs = ap_.tile([P, 1], F32, tag="rs")
                    nc.vector.reciprocal(rs[:], sm[:])
                    att = ap_.tile([P, P], F32, tag="att")
                    nc.vector.tensor_scalar_mul(att[:], e[:], rs[:])
                    paT = pp.tile([P, P], F32, tag="tpf")
                    nc.tensor.transpose(paT[:], att[:], id32[:])
                    attT = ap_.tile([P, P], F32, tag="attT")
                    nc.scalar.activation(attT[:], paT[:], COPY)
                    pz = pp2.tile([P, P], F32, tag="mm")
                    nc.tensor.matmul(pz[:], lhsT=ah[:], rhs=attT[:],
                                     start=True, stop=True)
                    zT = ap_.tile([P, P], F32, tag="zT")
                    nc.scalar.activation(zT[:], pz[:], COPY)
                    pah = pp2.tile([P, P], F32, tag="mm")
                    nc.tensor.matmul(pah[:], lhsT=zT[:], rhs=wb_t[:],
                                     start=True, stop=True)
                    rt = ap_.tile([P, H], F32, tag="rt")
                    nc.scalar.activation(rt[:], pah[:], RELU)
                    tot = ap_.tile([P, H], F32, tag="tot")
                    nc.vector.tensor_add(tot[:], rt[:], ah[:])
                    pmv = pp2.tile([MPB, H], F32, tag="pmv")
                    nc.tensor.matmul(pmv[:], lhsT=g_t[:], rhs=tot[:],
                                     start=True, stop=True)
                    nc.vector.tensor_copy(mv_g[:MPB, i * H:(i + 1) * H],
                                          pmv[:MPB, :])
                # mv rows for this group: g*AG*MPB .. +AG*MPB, MPB rows per block
                nc.sync.dma_start(
                    out=mv[g * AG * MPB:(g + 1) * AG * MPB, :].rearrange(
                        "(bb m) h -> m bb h", bb=AG, m=MPB),
                    in_=mv_g[:MPB, :AG * H].rearrange(
                        "m (bb h) -> m bb h", bb=AG))
            atom_phase(mf, final_group)
    nc.compile()
    return nc


def host_prep(f_atoms, f_bonds, W_i, W_h, W_o, b_o, W_a, W_b, b_b,
              a2b, b2a, b2revb, mol_size, A, B, AF, S):
    """Builds per-core in_maps."""
    As, Bs = A // NCORES, B // NCORES
    nblkA, nblkB = As // P, Bs // P
    ngA = nblkA // AG
    MPB = P // S

    W_op = np.concatenate([W_o[:133], b_o[None, :], W_o[133:]],
                          axis=0).astype(np.float16)
    fa_ext = np.concatenate([np.asarray(f_atoms, np.float32),
                             np.ones((A, 1), np.float32)], axis=1).astype(np.float16)
    fb16 = np.asarray(f_bonds, np.float16)
    amask = np.full((P, P), -30000.0, np.float32)
    for m in range(MPB):
        amask[m * S:(m + 1) * S, m * S:(m + 1) * S] = 0.0
    gsel = np.zeros((P, MPB), np.float32)
    for m in range(MPB):
        gsel[m * S:(m + 1) * S, m] = 1.0 / S

    common = dict(
        w_i=np.asarray(W_i, np.float16), w_h=np.asarray(W_h, np.float16),
        w_o=W_op, w_a=np.asarray(W_a, np.float32), w_b=np.asarray(W_b, np.float32),
        amask=amask, gsel=gsel,
    )
    # chunk-major m_full position map: global bond b -> chunk-major row
    NCH = 4
    csz = Bs // NCH
    ball = np.arange(B, dtype=np.int64)
    kk, oo = ball // Bs, ball % Bs
    cc, ww = oo // csz, oo % csz
    pos = (cc * (B // NCH) + kk * csz + ww).astype(np.int32)

    a2b_m = pos[a2b]                     # remapped into m_full chunk-major space
    b2revb_m = pos[b2revb]

    in_maps = []
    for k in range(NCORES):
        a0, b0 = k * As, k * Bs
        a2b_s = a2b_m[a0:a0 + As]        # [As, NB]
        # column layout per atom group g: col = j*AG + bb (j-major)
        idxA = np.ascontiguousarray(
            a2b_s.reshape(ngA, AG, P, NB).transpose(2, 0, 3, 1).reshape(
                P, ngA * NB * AG)).astype(np.int32)
        idxR = np.ascontiguousarray(
            b2revb_m[b0:b0 + Bs].reshape(nblkB, P).T).astype(np.int32)
        idxB = np.ascontiguousarray(
            b2a[b0:b0 + Bs].reshape(nblkB, P).T).astype(np.int32)
        in_maps.append(dict(
            fb=np.ascontiguousarray(fb16[b0:b0 + Bs]),
            fa=np.ascontiguousarray(fa_ext[a0:a0 + As]),
            idxA=idxA, idxR=idxR, idxB=idxB, **common))
    return in_maps


_NC_CACHE = {}


def get_nc(A, B, AF, S, no_cc=False, no_gather=False):
    key = (A, B, AF, S, no_cc, no_gather)
    if key not in _NC_CACHE:
        _NC_CACHE[key] = build_nc(A, B, AF, S, no_cc=no_cc, no_gather=no_gather)
    return _NC_CACHE[key]


def kernel(f_atoms, f_bonds, W_i, W_h, W_o, b_o, W_a, W_b, b_b,
           a2b, b2a, b2revb, mol_size):
    f_atoms = np.asarray(f_atoms, np.float32)
    f_bonds = np.asarray(f_bonds, np.float32)
    A, AF = f_atoms.shape
    B = f_bonds.shape[0]
    S = int(mol_size)
    nc = get_nc(A, B, AF, S)
    in_maps = host_prep(
        f_atoms, f_bonds, np.asarray(W_i), np.asarray(W_h), np.asarray(W_o),
        np.asarray(b_o), np.asarray(W_a), np.asarray(W_b), np.asarray(b_b),
        np.asarray(a2b), np.asarray(b2a), np.asarray(b2revb), S, A, B, AF, S)
    res = run_bass_kernel_spmd(nc, in_maps, core_ids=list(range(NCORES)))
    return np.concatenate([r["mv"] for r in res.results], axis=0)
